# revision 21
# baseline (speedup 1.0000x reference)
"""CorrelationLayer1D Trainium2 Bass kernel (v4).

Computes out[b, d, h, w] = sum_c x_1[b,c,h,w] * x2p[b,c,h,w+d] for d in [0, 41),
where x2p is x_2 width-padded by (8, 32).  Inputs [4,128,160,320] f32.

Sharding: data-parallel over H = 160 = 8*20 (correlation runs along W only, so
H-sharding needs no halo).  Per core, rows are processed in chunks of HC=10.

Structure per chunk (b, h-chunk):
 - inputs are cast f32->bf16 during the load DMA (SWDGE cast path on gpsimd).
 - PE computes per-row Grams in bf16 with M=32 stationary blocks, 4-way
   col-tiled via tile_position into [128|64, 72] PSUM tiles; partition p of a
   tile holds w-col 128*kgrp+p with its 41-wide band at cols (p%32)+d.
 - ScalarE copies each Gram tile into a bf16 atlas [wb, HC*72].
 - The per-partition band skew S[p,h,d] = G[p,h,(p%32)+d] is realized through
   a DRAM scratch round-trip (SBUF-side APs cannot shift per partition, DRAM
   APs can): the WRITE deskews - per 32-block bb the AP
   [[HC*72, 32],[72, HC],[1, 72]] at offset 32*(HC*72+1)*bb lands the band at
   the uniform address (HC*72+1)*p + 72*h + d - and the READ is then one plain
   2D full-plane AP [[HC*72+1, wb],[1, HC*72]] per group (1440B descriptors).
 - PE transposes each row's S [wb, 41] -> [41, wb] (bf16 identity matmul),
   VectorE copies (cast f32) into abatch [41, HC*320], one d-major store.

Engine streams form an explicit 2-deep software pipeline (engines execute
their streams in order, so cross-chunk work must interleave at emission):
  pool:   x1(k), x2(k) cast-loads + pad memsets, out-store(k-2)
  PE:     Gram matmuls(k), then transposes(k-1)
  scalar: PSUM->atlas copies(k)
  vector: transpose-PSUM->abatch copies(k-1)
  sync:   10 deskewing scratch writes(k), 3 full-plane reads(k)
The store runs on gpsimd/SWDGE because HWDGE pins all 41 descriptors of the
41-partition store onto a single SDMA engine (163us serial); SWDGE sprays
them across all 16.
"""

import sys

import numpy as np

try:
    import concourse.bass as bass  # noqa: F401
except ImportError:
    sys.path.insert(0, "/opt/trn_rl_repo")

import concourse.bass as bass
import concourse.tile as tile
from concourse import bacc, masks, mybir
from concourse.ap import AP
from concourse.bass_utils import run_bass_kernel_spmd

MAX_DISP = 40
D = MAX_DISP + 1  # 41 displacements
PAD_L = 8
PAD_R = 32
B, C, H, W = 4, 128, 160, 320
N_CORES = 8
HS = H // N_CORES  # 20 h-rows per core
WP = W + PAD_L + PAD_R  # 360
WGRPS = [128, 128, 64]  # partition-group widths covering W=320
GW = 32 + MAX_DISP  # compacted gram width per 32-col block: 72

F32 = mybir.dt.float32
BF16 = mybir.dt.bfloat16


def build_kernel(b_dim=B, hs=HS, hc=None):
    if hc is None:
        hc = 10 if hs % 10 == 0 else hs
    assert hs % hc == 0
    nchunks = hs // hc
    ALPHA = hc * GW + 1

    nc = bacc.Bacc("TRN2", target_bir_lowering=False, debug=False)
    x1e = nc.declare_dram_parameter("x1", [b_dim, C, hs, W], F32, isOutput=False)
    x2e = nc.declare_dram_parameter("x2", [b_dim, C, hs, W], F32, isOutput=False)
    oute = nc.declare_dram_parameter("out", [b_dim, D, hs, W], F32, isOutput=True)

    with tile.TileContext(nc) as tc:
        with (
            tc.tile_pool(name="const", bufs=1) as const_pool,
            tc.tile_pool(name="xin", bufs=4) as xin_pool,
            tc.tile_pool(name="atlas", bufs=3) as atlas_pool,
            tc.tile_pool(name="sbig", bufs=5) as sbig_pool,
            tc.tile_pool(name="asm", bufs=4) as asm_pool,
            tc.tile_pool(name="psum_g", bufs=5, space="PSUM") as psum_g,
            tc.tile_pool(name="psum_t", bufs=3, space="PSUM") as psum_t,
            tc.tile_pool(name="scratch", bufs=5, space="DRAM") as scratch_pool,
        ):
            identity = const_pool.tile([128, 128], BF16)
            masks.make_identity(nc, identity[:])

            def emit_loads(b, ci, k):
                h0 = ci * hc
                x1b = xin_pool.tile([C, hc * W], BF16, tag="x1b", name=f"x1b_{k}")
                nc.gpsimd.dma_start(
                    x1b[:].rearrange("p (h w) -> p h w", w=W),
                    x1e[b, :, h0 : h0 + hc, :],
                )
                x2b = xin_pool.tile([C, hc * WP], BF16, tag="x2b", name=f"x2b_{k}")
                x2b3 = x2b[:].rearrange("p (h w) -> p h w", w=WP)
                nc.gpsimd.memset(x2b3[:, :, 0:PAD_L], 0.0)
                nc.gpsimd.memset(x2b3[:, :, PAD_L + W : WP], 0.0)
                nc.gpsimd.dma_start(
                    x2b3[:, :, PAD_L : PAD_L + W], x2e[b, :, h0 : h0 + hc, :]
                )
                return x1b, x2b

            # Multi-row PSUM tiles: fewer, bigger ACT evacuations (ACT's
            # per-op overhead paced the Gram matmuls at 1 copy per row).
            rowgroups = []
            r0 = 0
            while r0 < hc:
                nr = min(4, hc - r0)
                rowgroups.append((r0, nr))
                r0 += nr

            def emit_grams(x1b, x2b, k):
                atlas = [
                    atlas_pool.tile([wb, hc * GW], BF16, tag=f"atl{g}", name=f"atl{g}_{k}")
                    for g, wb in enumerate(WGRPS)
                ]
                for r0, nr in rowgroups:
                    for kgrp, wb in enumerate(WGRPS):
                        gram_ps = psum_g.tile(
                            [wb, nr * GW], F32, tag="gram", name=f"gram_{k}_{r0}_{kgrp}"
                        )
                        for r in range(nr):
                            hh = r0 + r
                            o1 = hh * W
                            o2 = hh * WP
                            for kk in range(wb // 32):
                                w0 = 128 * kgrp + 32 * kk
                                nc.tensor.matmul(
                                    gram_ps[32 * kk : 32 * kk + 32, r * GW : (r + 1) * GW],
                                    x1b[:, o1 + w0 : o1 + w0 + 32],
                                    x2b[:, o2 + w0 : o2 + w0 + GW],
                                    start=True,
                                    stop=True,
                                    tile_position=(0, 32 * kk),
                                )
                        nc.scalar.copy(
                            atlas[kgrp][:, r0 * GW : (r0 + nr) * GW], gram_ps[:]
                        )
                return atlas

            def emit_roundtrip(atlas, k):
                # One deskewing write per group ((h,j) merges into one
                # contiguous 720-elem dim -> 3D AP) and merged reads: kgrp0+1
                # reload as a single [128, 1440] DMA, kgrp2 as [64, 720].
                PL = hc * GW  # 720: per-partition plane
                scrA = scratch_pool.tile(
                    [2 * 128 * ALPHA], BF16, tag="scrA", name=f"scrA_{k}"
                )
                scrB = scratch_pool.tile(
                    [64 * ALPHA], BF16, tag="scrB", name=f"scrB_{k}"
                )
                for kgrp, wb in enumerate(WGRPS):
                    scr_ap = scrA[:] if kgrp < 2 else scrB[:]
                    base = 128 * ALPHA if kgrp == 1 else 0
                    eng = nc.scalar if kgrp == 1 else nc.sync
                    dst = AP(
                        tensor=scr_ap.tensor,
                        offset=scr_ap.offset + base,
                        ap=[[32 * ALPHA, wb // 32], [PL, 32], [1, PL]],
                    )
                    eng.dma_start(dst, atlas[kgrp][:])
                # both reads on sync: a read's wait on write-completion must
                # not block ScalarE's stream (its atlas copies pace the Grams)
                sb01 = sbig_pool.tile([128, 2 * PL], BF16, tag="sb01", name=f"sb01_{k}")
                nc.sync.dma_start(
                    sb01[:].rearrange("p (g x) -> p g x", g=2),
                    AP(
                        tensor=scrA[:].tensor,
                        offset=scrA[:].offset,
                        ap=[[ALPHA, 128], [128 * ALPHA, 2], [1, PL]],
                    ),
                )
                sb2 = sbig_pool.tile([64, PL], BF16, tag="sb2", name=f"sb2_{k}")
                nc.sync.dma_start(
                    sb2[:],
                    AP(
                        tensor=scrB[:].tensor,
                        offset=scrB[:].offset,
                        ap=[[ALPHA, 64], [1, PL]],
                    ),
                )
                return (sb01, sb2)

            def emit_transposes(sbig, k):
                # All three group transposes of a row share one [41, 320] PSUM
                # tile (640B, fits a bank) -> one DVE evacuation per row.
                sb01, sb2 = sbig
                PL = hc * GW
                abatch = asm_pool.tile([D, hc * W], F32, tag="abatch", name=f"ab_{k}")
                for hh in range(hc):
                    t_ps = psum_t.tile([D, W], BF16, tag="t_ps", name=f"t_{k}_{hh}")
                    for kgrp, wb in enumerate(WGRPS):
                        src = (
                            sb01[:, kgrp * PL + hh * GW : kgrp * PL + hh * GW + D]
                            if kgrp < 2
                            else sb2[:, hh * GW : hh * GW + D]
                        )
                        nc.tensor.matmul(
                            t_ps[:, 128 * kgrp : 128 * kgrp + wb],
                            src,
                            identity[0:wb, 0:wb],
                            start=True,
                            stop=True,
                            is_transpose=True,
                        )
                    nc.vector.tensor_copy(
                        abatch[:, hh * W : (hh + 1) * W], t_ps[:]
                    )
                return abatch

            def emit_out(st):
                b, ci, abatch = st["b"], st["ci"], st["abatch"]
                h0 = ci * hc
                nc.gpsimd.dma_start(
                    oute[b, :, h0 : h0 + hc, :],
                    abatch[:].rearrange("d (h w) -> d h w", w=W),
                )

            # 3-deep software pipeline: loads prefetch 1 chunk ahead, the
            # scratch round-trip of chunk k overlaps Grams of k+1, transposes
            # of k run after Grams of k+2, stores trail at depth 3.
            seq = [(b, ci) for b in range(b_dim) for ci in range(nchunks)]
            n = len(seq)
            pend = {}
            loaded = {}
            for k, (b, ci) in enumerate(seq):
                if k == 0:
                    loaded[0] = emit_loads(*seq[0], 0)
                    if n > 1:
                        loaded[1] = emit_loads(*seq[1], 1)
                elif k + 1 < n:
                    loaded[k + 1] = emit_loads(*seq[k + 1], k + 1)
                if k >= 4:
                    emit_out(pend.pop(k - 4))
                x1b, x2b = loaded.pop(k)
                atlas = emit_grams(x1b, x2b, k)
                if k >= 3:
                    pend[k - 3]["abatch"] = emit_transposes(pend[k - 3]["sbig"], k - 3)
                sbig = emit_roundtrip(atlas, k)
                pend[k] = {"b": b, "ci": ci, "sbig": sbig}
            # drain
            for k in range(max(0, n - 3), n):
                if "abatch" not in pend[k]:
                    pend[k]["abatch"] = emit_transposes(pend[k]["sbig"], k)
            for k in range(max(0, n - 4), n):
                if k in pend:
                    emit_out(pend.pop(k))

    nc.finalize()
    return nc


_compiled = {}


def _get_kernel(b_dim, hs):
    key = (b_dim, hs)
    if key not in _compiled:
        _compiled[key] = build_kernel(b_dim, hs)
    return _compiled[key]


def kernel(x_1: np.ndarray, x_2: np.ndarray) -> np.ndarray:
    assert x_1.shape == (B, C, H, W) and x_2.shape == (B, C, H, W)
    x_1 = np.ascontiguousarray(x_1, dtype=np.float32)
    x_2 = np.ascontiguousarray(x_2, dtype=np.float32)
    nc = _get_kernel(B, HS)
    in_maps = [
        {
            "x1": np.ascontiguousarray(x_1[:, :, i * HS : (i + 1) * HS, :]),
            "x2": np.ascontiguousarray(x_2[:, :, i * HS : (i + 1) * HS, :]),
        }
        for i in range(N_CORES)
    ]
    res = run_bass_kernel_spmd(nc, in_maps, core_ids=list(range(N_CORES))).results
    out = np.concatenate([res[i]["out"] for i in range(N_CORES)], axis=2)
    return out


# revision 24
# speedup vs baseline: 1.0206x; 1.0206x over previous
"""CorrelationLayer1D Trainium2 Bass kernel (v4).

Computes out[b, d, h, w] = sum_c x_1[b,c,h,w] * x2p[b,c,h,w+d] for d in [0, 41),
where x2p is x_2 width-padded by (8, 32).  Inputs [4,128,160,320] f32.

Sharding: data-parallel over H = 160 = 8*20 (correlation runs along W only, so
H-sharding needs no halo).  Per core, rows are processed in chunks of HC=10.

Structure per chunk (b, h-chunk):
 - inputs are cast f32->bf16 during the load DMA (SWDGE cast path on gpsimd).
 - PE computes per-row Grams in bf16 with M=32 stationary blocks, 4-way
   col-tiled via tile_position into [128|64, 72] PSUM tiles; partition p of a
   tile holds w-col 128*kgrp+p with its 41-wide band at cols (p%32)+d.
 - ScalarE copies each Gram tile into a bf16 atlas [wb, HC*72].
 - The per-partition band skew S[p,h,d] = G[p,h,(p%32)+d] is realized through
   a DRAM scratch round-trip (SBUF-side APs cannot shift per partition, DRAM
   APs can): the WRITE deskews - per 32-block bb the AP
   [[HC*72, 32],[72, HC],[1, 72]] at offset 32*(HC*72+1)*bb lands the band at
   the uniform address (HC*72+1)*p + 72*h + d - and the READ is then one plain
   2D full-plane AP [[HC*72+1, wb],[1, HC*72]] per group (1440B descriptors).
 - PE transposes each row's S [wb, 41] -> [41, wb] (bf16 identity matmul),
   VectorE copies (cast f32) into abatch [41, HC*320], one d-major store.

Engine streams form an explicit 2-deep software pipeline (engines execute
their streams in order, so cross-chunk work must interleave at emission):
  pool:   x1(k), x2(k) cast-loads + pad memsets, out-store(k-2)
  PE:     Gram matmuls(k), then transposes(k-1)
  scalar: PSUM->atlas copies(k)
  vector: transpose-PSUM->abatch copies(k-1)
  sync:   10 deskewing scratch writes(k), 3 full-plane reads(k)
The store runs on gpsimd/SWDGE because HWDGE pins all 41 descriptors of the
41-partition store onto a single SDMA engine (163us serial); SWDGE sprays
them across all 16.
"""

import sys

import numpy as np

try:
    import concourse.bass as bass  # noqa: F401
except ImportError:
    sys.path.insert(0, "/opt/trn_rl_repo")

import concourse.bass as bass
import concourse.tile as tile
from concourse import bacc, masks, mybir
from concourse.ap import AP
from concourse.bass_utils import run_bass_kernel_spmd

MAX_DISP = 40
D = MAX_DISP + 1  # 41 displacements
PAD_L = 8
PAD_R = 32
B, C, H, W = 4, 128, 160, 320
N_CORES = 8
HS = H // N_CORES  # 20 h-rows per core
WP = W + PAD_L + PAD_R  # 360
WGRPS = [128, 128, 64]  # partition-group widths covering W=320
GW = 32 + MAX_DISP  # compacted gram width per 32-col block: 72

F32 = mybir.dt.float32
BF16 = mybir.dt.bfloat16


def build_kernel(b_dim=B, hs=HS, hc=None):
    if hc is None:
        hc = 10 if hs % 10 == 0 else hs
    assert hs % hc == 0
    nchunks = hs // hc
    ALPHA = hc * GW + 1

    nc = bacc.Bacc("TRN2", target_bir_lowering=False, debug=False)
    x1e = nc.declare_dram_parameter("x1", [b_dim, C, hs, W], F32, isOutput=False)
    x2e = nc.declare_dram_parameter("x2", [b_dim, C, hs, W], F32, isOutput=False)
    oute = nc.declare_dram_parameter("out", [b_dim, D, hs, W], F32, isOutput=True)

    with tile.TileContext(nc) as tc:
        with (
            tc.tile_pool(name="const", bufs=1) as const_pool,
            tc.tile_pool(name="xin", bufs=4) as xin_pool,
            tc.tile_pool(name="atlas", bufs=3) as atlas_pool,
            tc.tile_pool(name="sbig", bufs=5) as sbig_pool,
            tc.tile_pool(name="asm", bufs=4) as asm_pool,
            tc.tile_pool(name="psum_g", bufs=5, space="PSUM") as psum_g,
            tc.tile_pool(name="psum_t", bufs=3, space="PSUM") as psum_t,
            tc.tile_pool(name="scratch", bufs=5, space="DRAM") as scratch_pool,
        ):
            identity = const_pool.tile([128, 128], BF16)
            masks.make_identity(nc, identity[:])

            def emit_loads(b, ci, k):
                h0 = ci * hc
                x1b = xin_pool.tile([C, hc * W], BF16, tag="x1b", name=f"x1b_{k}")
                nc.gpsimd.dma_start(
                    x1b[:].rearrange("p (h w) -> p h w", w=W),
                    x1e[b, :, h0 : h0 + hc, :],
                )
                x2b = xin_pool.tile([C, hc * WP], BF16, tag="x2b", name=f"x2b_{k}")
                x2b3 = x2b[:].rearrange("p (h w) -> p h w", w=WP)
                nc.gpsimd.memset(x2b3[:, :, 0:PAD_L], 0.0)
                nc.gpsimd.memset(x2b3[:, :, PAD_L + W : WP], 0.0)
                nc.gpsimd.dma_start(
                    x2b3[:, :, PAD_L : PAD_L + W], x2e[b, :, h0 : h0 + hc, :]
                )
                return x1b, x2b

            # Multi-row PSUM tiles: fewer, bigger ACT evacuations (ACT's
            # per-op overhead paced the Gram matmuls at 1 copy per row).
            rowgroups = []
            r0 = 0
            while r0 < hc:
                nr = min(4, hc - r0)
                rowgroups.append((r0, nr))
                r0 += nr

            def emit_grams(x1b, x2b, k):
                atlas = [
                    atlas_pool.tile([wb, hc * GW], BF16, tag=f"atl{g}", name=f"atl{g}_{k}")
                    for g, wb in enumerate(WGRPS)
                ]
                for r0, nr in rowgroups:
                    for kgrp, wb in enumerate(WGRPS):
                        gram_ps = psum_g.tile(
                            [wb, nr * GW], F32, tag="gram", name=f"gram_{k}_{r0}_{kgrp}"
                        )
                        for r in range(nr):
                            hh = r0 + r
                            o1 = hh * W
                            o2 = hh * WP
                            for kk in range(wb // 32):
                                w0 = 128 * kgrp + 32 * kk
                                nc.tensor.matmul(
                                    gram_ps[32 * kk : 32 * kk + 32, r * GW : (r + 1) * GW],
                                    x1b[:, o1 + w0 : o1 + w0 + 32],
                                    x2b[:, o2 + w0 : o2 + w0 + GW],
                                    start=True,
                                    stop=True,
                                    tile_position=(0, 32 * kk),
                                )
                        nc.scalar.copy(
                            atlas[kgrp][:, r0 * GW : (r0 + nr) * GW], gram_ps[:]
                        )
                return atlas

            def emit_roundtrip(atlas, k):
                # One deskewing write per group ((h,j) merges into one
                # contiguous 720-elem dim -> 3D AP [[32A, nb],[720, 32],[1, 720]]),
                # then three plain reads.  All writes are issued before any
                # read so the sync sequencer never serializes two full
                # write->wait->read round trips; r1's wait on the scalar-issued
                # w1 comes last, by which time w1 has long completed.
                PL = hc * GW  # 720: per-partition plane
                scrA = scratch_pool.tile(
                    [2 * 128 * ALPHA], BF16, tag="scrA", name=f"scrA_{k}"
                )
                scrB = scratch_pool.tile(
                    [64 * ALPHA], BF16, tag="scrB", name=f"scrB_{k}"
                )
                for kgrp, wb in enumerate(WGRPS):
                    scr_ap = scrA[:] if kgrp < 2 else scrB[:]
                    base = 128 * ALPHA if kgrp == 1 else 0
                    eng = nc.scalar if kgrp == 1 else nc.sync
                    dst = AP(
                        tensor=scr_ap.tensor,
                        offset=scr_ap.offset + base,
                        ap=[[32 * ALPHA, wb // 32], [PL, 32], [1, PL]],
                    )
                    eng.dma_start(dst, atlas[kgrp][:])
                sbig = [None, None, None]
                for kgrp in (0, 2, 1):  # r1 (waits cross-engine w1) goes last
                    wb = WGRPS[kgrp]
                    scr_ap = scrA[:] if kgrp < 2 else scrB[:]
                    base = 128 * ALPHA if kgrp == 1 else 0
                    sb = sbig_pool.tile(
                        [wb, PL], BF16, tag=f"sb{kgrp}", name=f"sb{kgrp}_{k}"
                    )
                    nc.sync.dma_start(
                        sb[:],
                        AP(
                            tensor=scr_ap.tensor,
                            offset=scr_ap.offset + base,
                            ap=[[ALPHA, wb], [1, PL]],
                        ),
                    )
                    sbig[kgrp] = sb
                return sbig

            def emit_transposes(sbig, k):
                # All three group transposes of a row share one [41, 320] PSUM
                # tile (640B, fits a bank) -> one DVE evacuation per row.
                abatch = asm_pool.tile([D, hc * W], F32, tag="abatch", name=f"ab_{k}")
                for hh in range(hc):
                    t_ps = psum_t.tile([D, W], BF16, tag="t_ps", name=f"t_{k}_{hh}")
                    for kgrp, wb in enumerate(WGRPS):
                        nc.tensor.matmul(
                            t_ps[:, 128 * kgrp : 128 * kgrp + wb],
                            sbig[kgrp][:, hh * GW : hh * GW + D],
                            identity[0:wb, 0:wb],
                            start=True,
                            stop=True,
                            is_transpose=True,
                        )
                    nc.vector.tensor_copy(
                        abatch[:, hh * W : (hh + 1) * W], t_ps[:]
                    )
                return abatch

            def emit_out(st):
                b, ci, abatch = st["b"], st["ci"], st["abatch"]
                h0 = ci * hc
                nc.gpsimd.dma_start(
                    oute[b, :, h0 : h0 + hc, :],
                    abatch[:].rearrange("d (h w) -> d h w", w=W),
                )

            # 3-deep software pipeline: loads prefetch 1 chunk ahead, the
            # scratch round-trip of chunk k overlaps Grams of k+1, transposes
            # of k run after Grams of k+2, stores trail at depth 3.
            seq = [(b, ci) for b in range(b_dim) for ci in range(nchunks)]
            n = len(seq)
            pend = {}
            loaded = {}
            for k, (b, ci) in enumerate(seq):
                if k == 0:
                    loaded[0] = emit_loads(*seq[0], 0)
                    if n > 1:
                        loaded[1] = emit_loads(*seq[1], 1)
                elif k + 1 < n:
                    loaded[k + 1] = emit_loads(*seq[k + 1], k + 1)
                if k >= 4:
                    emit_out(pend.pop(k - 4))
                x1b, x2b = loaded.pop(k)
                atlas = emit_grams(x1b, x2b, k)
                if k >= 3:
                    pend[k - 3]["abatch"] = emit_transposes(pend[k - 3]["sbig"], k - 3)
                sbig = emit_roundtrip(atlas, k)
                pend[k] = {"b": b, "ci": ci, "sbig": sbig}
            # drain
            for k in range(max(0, n - 3), n):
                if "abatch" not in pend[k]:
                    pend[k]["abatch"] = emit_transposes(pend[k]["sbig"], k)
            for k in range(max(0, n - 4), n):
                if k in pend:
                    emit_out(pend.pop(k))

    nc.finalize()
    return nc


_compiled = {}


def _get_kernel(b_dim, hs):
    key = (b_dim, hs)
    if key not in _compiled:
        _compiled[key] = build_kernel(b_dim, hs)
    return _compiled[key]


def kernel(x_1: np.ndarray, x_2: np.ndarray) -> np.ndarray:
    assert x_1.shape == (B, C, H, W) and x_2.shape == (B, C, H, W)
    x_1 = np.ascontiguousarray(x_1, dtype=np.float32)
    x_2 = np.ascontiguousarray(x_2, dtype=np.float32)
    nc = _get_kernel(B, HS)
    in_maps = [
        {
            "x1": np.ascontiguousarray(x_1[:, :, i * HS : (i + 1) * HS, :]),
            "x2": np.ascontiguousarray(x_2[:, :, i * HS : (i + 1) * HS, :]),
        }
        for i in range(N_CORES)
    ]
    res = run_bass_kernel_spmd(nc, in_maps, core_ids=list(range(N_CORES))).results
    out = np.concatenate([res[i]["out"] for i in range(N_CORES)], axis=2)
    return out
